# revision 9
# baseline (speedup 1.0000x reference)
"""LSTM layer kernel for Trainium2 (8 NeuronCores, batch-sharded).

Problem: data [64, 2048, 128] f32, W [256, 512] f32, b [512] f32.
  xp = data @ W[:128] + b   (hoisted input projection)
  per step: z = xp_t + h @ W[128:]; i,f,o,g = split(z,4)
            c = sig(f)*c + sig(i)*tanh(g); h = sig(o)*tanh(c)
  h0 = 0, c0 = 1.  Output: all h, [64, 2048, 128] f32.

Design (per core, batch shard of 8 sequences):
  - State layout transposed: [128 units (partitions), 8 batch (free)].
  - PSUM bank per 16-step block holds z laid out [128, (t, gate, batch)];
    input projection is matmul-ed into the bank ahead of time (start=True
    group), the 4 per-step recurrent matmuls accumulate onto it.
  - tanh(x) = 2*sigmoid(2x) - 1: the x2 is folded into the g-gate columns
    of W host-side, so ONE sigmoid ACT op covers all 4 gates per step.
  - Elementwise per step: m = (sg*2-1)*si; c = c*sf; c = c+m;
    sc = sigmoid(2c); h = (sc*2-1)*so   (affine_mul_reduce fuses ax+b mul).
  - x transposed on-chip via PE transpose (fp32 has no DMA transpose);
    h transposed back per block and DMA-ed out.
"""

import os
import sys

sys.path.insert(0, "/opt/trn_rl_repo")

import numpy as np

import concourse.bacc as bacc
import concourse.mybir as mybir
import concourse.tile as tile
from concourse import bass_utils
from concourse.masks import make_identity

B, T, D, U = 64, 2048, 128, 128
NCORES = 8
BSH = B // NCORES          # 8 sequences per core
TB = 16                    # time steps per psum-bank block
NBLK = T // TB
F32 = mybir.dt.float32
SIG = mybir.ActivationFunctionType.Sigmoid


def _build(with_bias: bool):
    nc = bacc.Bacc("TRN2", target_bir_lowering=False, debug=False,
                   num_devices=NCORES)
    data_t = nc.dram_tensor("data", [BSH, T, D], F32, kind="ExternalInput")
    wx_t = nc.dram_tensor("wx", [D, 4 * U], F32, kind="ExternalInput")
    wh_t = nc.dram_tensor("wh", [U, 4 * U], F32, kind="ExternalInput")
    if with_bias:
        bmat_t = nc.dram_tensor("bmat", [4, U], F32, kind="ExternalInput")
        bsel_t = nc.dram_tensor("bsel", [4, TB * 4 * BSH], F32,
                                kind="ExternalInput")
    out_t = nc.dram_tensor("out", [BSH, T, U], F32, kind="ExternalOutput")

    data_ap = data_t.ap()
    out_ap = out_t.ap()

    with tile.TileContext(nc) as tc:
        with (
            tc.tile_pool(name="const", bufs=1) as constp,
            tc.tile_pool(name="xnat", bufs=3) as xnatp,
            tc.tile_pool(name="xt", bufs=3) as xtp,
            tc.tile_pool(name="hst", bufs=3) as hstp,
            tc.tile_pool(name="hnat", bufs=2) as hnatp,
            tc.tile_pool(name="s", bufs=3) as sp,
            tc.tile_pool(name="small", bufs=3) as smallp,
            tc.tile_pool(name="zb", bufs=2, space="PSUM") as zbp,
            tc.tile_pool(name="xps", bufs=2, space="PSUM") as xpsp,
            tc.tile_pool(name="hps", bufs=2, space="PSUM") as hpsp,
        ):
            wx = constp.tile([D, 4 * U], F32, tag="wx")
            wh = constp.tile([U, 4 * U], F32, tag="wh")
            ident = constp.tile([128, 128], F32, tag="ident")
            c = constp.tile([U, BSH], F32, tag="c")
            acc1 = constp.tile([128, 1], F32, tag="acc1")
            acc2 = constp.tile([128, 1], F32, tag="acc2")
            nc.sync.dma_start(wx[:], wx_t.ap())
            nc.sync.dma_start(wh[:], wh_t.ap())
            make_identity(nc, ident[:])
            nc.vector.memset(c[:], 1.0)
            if with_bias:
                bmat = constp.tile([4, U], F32, tag="bmat")
                bsel = constp.tile([4, TB * 4 * BSH], F32, tag="bsel")
                nc.sync.dma_start(bmat[:], bmat_t.ap())
                nc.sync.dma_start(bsel[:], bsel_t.ap())

            h_prev = None
            for kb in range(NBLK):
                t0 = kb * TB
                # ---- input projection for this block ----
                x_nat = xnatp.tile([TB * BSH, D], F32, tag="xnat")
                for bb in range(BSH):
                    nc.sync.dma_start(
                        x_nat[bb * TB:(bb + 1) * TB, :],
                        data_ap[bb, t0:t0 + TB, :],
                    )
                x_ps = xpsp.tile([D, TB * BSH], F32, tag="xps")
                nc.tensor.transpose(x_ps[:], x_nat[:], ident[:])
                xT = xtp.tile([D, TB * BSH], F32, tag="xt")
                nc.vector.tensor_copy(xT[:], x_ps[:])
                # xT columns are (b, t); stream them (t outer, b inner) to
                # match the psum layout [U, t, g, b]
                xT_tb = xT[:].rearrange("p (b t) -> p t b", b=BSH)

                zb = zbp.tile([U, TB, 4, BSH], F32, tag="zb")
                for g in range(4):
                    nc.tensor.matmul(
                        zb[:, :, g, :],
                        lhsT=wx[:, g * U:(g + 1) * U],
                        rhs=xT_tb,
                        start=(g == 0),
                        stop=False,
                    )
                if with_bias:
                    nc.tensor.matmul(
                        zb[:, :, :, :],
                        lhsT=bmat[:],
                        rhs=bsel[:],
                        start=False,
                        stop=False,
                    )

                # hsT columns are (b, t) so the block-end transpose rows come
                # out (b, t)-ordered for a contiguous store; per-step slices
                # are therefore strided (step TB, count BSH).
                hsT = hstp.tile([U, BSH, TB], F32, tag="hst")

                # ---- sequential steps ----
                for tl in range(TB):
                    t = t0 + tl
                    if t > 0:
                        for g in range(4):
                            nc.tensor.matmul(
                                zb[:, tl, g, :],
                                lhsT=wh[:, g * U:(g + 1) * U],
                                rhs=h_prev,
                                start=False,
                                stop=(g == 3),
                            )
                    s = sp.tile([U, 4 * BSH], F32, tag="s")
                    nc.scalar.activation(s[:], zb[:, tl, :, :], SIG)
                    si = s[:, 0 * BSH:1 * BSH]
                    sf = s[:, 1 * BSH:2 * BSH]
                    so = s[:, 2 * BSH:3 * BSH]
                    sg = s[:, 3 * BSH:4 * BSH]
                    m = smallp.tile([U, BSH], F32, tag="m")
                    nc.vector.affine_mul_reduce(m[:], acc1[:], sg, si, 2.0, -1.0)
                    nc.vector.tensor_mul(c[:], c[:], sf)
                    nc.vector.tensor_add(c[:], c[:], m[:])
                    sc = smallp.tile([U, BSH], F32, tag="sc")
                    nc.scalar.activation(sc[:], c[:], SIG, scale=2.0)
                    h_slice = hsT[:, :, tl]
                    nc.vector.affine_mul_reduce(h_slice, acc2[:], sc[:], so,
                                                2.0, -1.0)
                    h_prev = h_slice

                # ---- write block output ----
                h_ps = hpsp.tile([TB * BSH, U], F32, tag="hps")
                nc.tensor.transpose(h_ps[:], hsT[:, :, :].rearrange(
                    "p b t -> p (b t)"), ident[:])
                h_nat = hnatp.tile([TB * BSH, U], F32, tag="hnat")
                nc.vector.tensor_copy(h_nat[:], h_ps[:])
                for bb in range(BSH):
                    nc.sync.dma_start(
                        out_ap[bb, t0:t0 + TB, :],
                        h_nat[bb * TB:(bb + 1) * TB, :],
                    )

    nc.compile()
    return nc


def _prep_weights(W: np.ndarray, b: np.ndarray):
    Wp = W.astype(np.float32).copy()
    bp = b.astype(np.float32).copy()
    Wp[:, 3 * U:] *= 2.0          # fold tanh(x)=2*sig(2x)-1 into g gate
    bp[3 * U:] *= 2.0
    return Wp[:D], Wp[D:], bp


def run(data, W, b, trace=False):
    assert data.shape == (B, T, D), data.shape
    assert W.shape == (D + U, 4 * U), W.shape
    assert b.shape == (4 * U,), b.shape
    wx, wh, bp = _prep_weights(np.asarray(W), np.asarray(b))
    data = np.ascontiguousarray(np.asarray(data, dtype=np.float32))
    with_bias = bool(np.any(bp != 0.0))

    nc = _build(with_bias)

    in_maps = []
    for cid in range(NCORES):
        m = {
            "data": data[cid * BSH:(cid + 1) * BSH],
            "wx": np.ascontiguousarray(wx),
            "wh": np.ascontiguousarray(wh),
        }
        if with_bias:
            # bmat[gp, u] = b[gp*U + u]; bsel[gp, (t, g, bb)] = (g == gp)
            bmat = bp.reshape(4, U)
            bsel = np.zeros((4, TB, 4, BSH), dtype=np.float32)
            for gp in range(4):
                bsel[gp, :, gp, :] = 1.0
            m["bmat"] = np.ascontiguousarray(bmat)
            m["bsel"] = bsel.reshape(4, TB * 4 * BSH)
        in_maps.append(m)

    res = bass_utils.run_bass_kernel_spmd(
        nc, in_maps, core_ids=list(range(NCORES)), trace=trace,
    )
    out = np.concatenate([res.results[cid]["out"] for cid in range(NCORES)],
                         axis=0)
    return out, res


def kernel(data, W, b):
    out, _ = run(data, W, b, trace=False)
    return out


# revision 10
# speedup vs baseline: 35.9470x; 35.9470x over previous
"""LSTM layer kernel for Trainium2 (8 NeuronCores, batch-sharded).

Problem: data [64, 2048, 128] f32, W [256, 512] f32, b [512] f32.
  xp = data @ W[:128] + b   (hoisted input projection)
  per step: z = xp_t + h @ W[128:]; i,f,o,g = split(z,4)
            c = sig(f)*c + sig(i)*tanh(g); h = sig(o)*tanh(c)
  h0 = 0, c0 = 1.  Output: all h, [64, 2048, 128] f32.

Design (per core, batch shard of 8 sequences):
  - State layout transposed: [128 units (partitions), 8 batch (free)].
  - PSUM bank per 16-step block holds z laid out [128, (t, gate, batch)];
    input projection is matmul-ed into the bank ahead of time (start=True
    group), the 4 per-step recurrent matmuls accumulate onto it.
  - tanh(x) = 2*sigmoid(2x) - 1: the x2 is folded into the g-gate columns
    of W host-side, so ONE sigmoid ACT op covers all 4 gates per step.
  - Elementwise per step: m = (sg*2-1)*si; c = c*sf; c = c+m;
    sc = sigmoid(2c); h = (sc*2-1)*so   (affine_mul_reduce fuses ax+b mul).
  - x transposed on-chip via PE transpose (fp32 has no DMA transpose);
    h transposed back per block and DMA-ed out.
"""

import os
import sys

sys.path.insert(0, "/opt/trn_rl_repo")

import numpy as np

import concourse.bacc as bacc
import concourse.mybir as mybir
import concourse.tile as tile
from concourse import bass_utils
from concourse.masks import make_identity

B, T, D, U = 64, 2048, 128, 128
NCORES = 8
BSH = B // NCORES          # 8 sequences per core
TB = 16                    # time steps per psum-bank block
NBLK = T // TB
F32 = mybir.dt.float32
SIG = mybir.ActivationFunctionType.Sigmoid


def _build(with_bias: bool, T: int = T, NBLK: int = NBLK):
    nc = bacc.Bacc("TRN2", target_bir_lowering=False, debug=False,
                   num_devices=NCORES)
    data_t = nc.dram_tensor("data", [BSH, T, D], F32, kind="ExternalInput")
    wx_t = nc.dram_tensor("wx", [D, 4 * U], F32, kind="ExternalInput")
    wh_t = nc.dram_tensor("wh", [U, 4 * U], F32, kind="ExternalInput")
    if with_bias:
        bmat_t = nc.dram_tensor("bmat", [4, U], F32, kind="ExternalInput")
        bsel_t = nc.dram_tensor("bsel", [4, TB * 4 * BSH], F32,
                                kind="ExternalInput")
    out_t = nc.dram_tensor("out", [BSH, T, U], F32, kind="ExternalOutput")

    data_ap = data_t.ap()
    out_ap = out_t.ap()

    with tile.TileContext(nc) as tc:
        with (
            tc.tile_pool(name="const", bufs=1) as constp,
            tc.tile_pool(name="xnat", bufs=3) as xnatp,
            tc.tile_pool(name="xt", bufs=3) as xtp,
            tc.tile_pool(name="hst", bufs=3) as hstp,
            tc.tile_pool(name="hnat", bufs=2) as hnatp,
            tc.tile_pool(name="s", bufs=3) as sp,
            tc.tile_pool(name="small", bufs=3) as smallp,
            tc.tile_pool(name="zb", bufs=2, space="PSUM") as zbp,
            tc.tile_pool(name="xps", bufs=2, space="PSUM") as xpsp,
            tc.tile_pool(name="hps", bufs=2, space="PSUM") as hpsp,
        ):
            wx = constp.tile([D, 4 * U], F32, tag="wx")
            wh = constp.tile([U, 4 * U], F32, tag="wh")
            ident = constp.tile([128, 128], F32, tag="ident")
            c = constp.tile([U, BSH], F32, tag="c")
            acc1 = constp.tile([128, 1], F32, tag="acc1")
            acc2 = constp.tile([128, 1], F32, tag="acc2")
            nc.sync.dma_start(wx[:], wx_t.ap())
            nc.sync.dma_start(wh[:], wh_t.ap())
            make_identity(nc, ident[:])
            nc.vector.memset(c[:], 1.0)
            if with_bias:
                bmat = constp.tile([4, U], F32, tag="bmat")
                bsel = constp.tile([4, TB * 4 * BSH], F32, tag="bsel")
                nc.sync.dma_start(bmat[:], bmat_t.ap())
                nc.sync.dma_start(bsel[:], bsel_t.ap())

            h_prev = None
            for kb in range(NBLK):
                t0 = kb * TB
                # ---- input projection for this block ----
                x_nat = xnatp.tile([TB * BSH, D], F32, tag="xnat")
                for bb in range(BSH):
                    nc.sync.dma_start(
                        x_nat[bb * TB:(bb + 1) * TB, :],
                        data_ap[bb, t0:t0 + TB, :],
                    )
                x_ps = xpsp.tile([D, TB * BSH], F32, tag="xps")
                nc.tensor.transpose(x_ps[:], x_nat[:], ident[:])
                xT = xtp.tile([D, TB * BSH], F32, tag="xt")
                nc.vector.tensor_copy(xT[:], x_ps[:])
                # xT columns are (b, t); stream them (t outer, b inner) to
                # match the psum layout [U, t, g, b]
                xT_tb = xT[:].rearrange("p (b t) -> p t b", b=BSH)

                zb = zbp.tile([U, TB, 4, BSH], F32, tag="zb")
                for g in range(4):
                    nc.tensor.matmul(
                        zb[:, :, g, :],
                        lhsT=wx[:, g * U:(g + 1) * U],
                        rhs=xT_tb,
                        start=(g == 0),
                        stop=False,
                    )
                if with_bias:
                    nc.tensor.matmul(
                        zb[:, :, :, :],
                        lhsT=bmat[:],
                        rhs=bsel[:],
                        start=False,
                        stop=False,
                    )

                # hsT columns are (b, t) so the block-end transpose rows come
                # out (b, t)-ordered for a contiguous store; per-step slices
                # are therefore strided (step TB, count BSH).
                hsT = hstp.tile([U, BSH, TB], F32, tag="hst")

                # ---- sequential steps ----
                for tl in range(TB):
                    t = t0 + tl
                    if t > 0:
                        for g in range(4):
                            nc.tensor.matmul(
                                zb[:, tl, g, :],
                                lhsT=wh[:, g * U:(g + 1) * U],
                                rhs=h_prev,
                                start=False,
                                stop=(g == 3),
                            )
                    s = sp.tile([U, 4 * BSH], F32, tag="s")
                    nc.scalar.activation(s[:], zb[:, tl, :, :], SIG)
                    si = s[:, 0 * BSH:1 * BSH]
                    sf = s[:, 1 * BSH:2 * BSH]
                    so = s[:, 2 * BSH:3 * BSH]
                    sg = s[:, 3 * BSH:4 * BSH]
                    m = smallp.tile([U, BSH], F32, tag="m")
                    nc.vector.affine_mul_reduce(m[:], acc1[:], sg, si, 2.0, -1.0)
                    nc.vector.tensor_mul(c[:], c[:], sf)
                    nc.vector.tensor_add(c[:], c[:], m[:])
                    sc = smallp.tile([U, BSH], F32, tag="sc")
                    nc.scalar.activation(sc[:], c[:], SIG, scale=2.0)
                    h_slice = hsT[:, :, tl]
                    nc.vector.affine_mul_reduce(h_slice, acc2[:], sc[:], so,
                                                2.0, -1.0)
                    h_prev = h_slice

                # ---- write block output ----
                h_ps = hpsp.tile([TB * BSH, U], F32, tag="hps")
                nc.tensor.transpose(h_ps[:], hsT[:, :, :].rearrange(
                    "p b t -> p (b t)"), ident[:])
                h_nat = hnatp.tile([TB * BSH, U], F32, tag="hnat")
                nc.vector.tensor_copy(h_nat[:], h_ps[:])
                for bb in range(BSH):
                    nc.sync.dma_start(
                        out_ap[bb, t0:t0 + TB, :],
                        h_nat[bb * TB:(bb + 1) * TB, :],
                    )

    nc.compile()
    return nc


def _prep_weights(W: np.ndarray, b: np.ndarray):
    Wp = W.astype(np.float32).copy()
    bp = b.astype(np.float32).copy()
    Wp[:, 3 * U:] *= 2.0          # fold tanh(x)=2*sig(2x)-1 into g gate
    bp[3 * U:] *= 2.0
    return Wp[:D], Wp[D:], bp


def run(data, W, b, trace=False):
    assert data.shape == (B, T, D), data.shape
    assert W.shape == (D + U, 4 * U), W.shape
    assert b.shape == (4 * U,), b.shape
    wx, wh, bp = _prep_weights(np.asarray(W), np.asarray(b))
    data = np.ascontiguousarray(np.asarray(data, dtype=np.float32))
    with_bias = bool(np.any(bp != 0.0))

    nc = _build(with_bias)

    in_maps = []
    for cid in range(NCORES):
        m = {
            "data": data[cid * BSH:(cid + 1) * BSH],
            "wx": np.ascontiguousarray(wx),
            "wh": np.ascontiguousarray(wh),
        }
        if with_bias:
            # bmat[gp, u] = b[gp*U + u]; bsel[gp, (t, g, bb)] = (g == gp)
            bmat = bp.reshape(4, U)
            bsel = np.zeros((4, TB, 4, BSH), dtype=np.float32)
            for gp in range(4):
                bsel[gp, :, gp, :] = 1.0
            m["bmat"] = np.ascontiguousarray(bmat)
            m["bsel"] = bsel.reshape(4, TB * 4 * BSH)
        in_maps.append(m)

    res = bass_utils.run_bass_kernel_spmd(
        nc, in_maps, core_ids=list(range(NCORES)), trace=trace,
    )
    out = np.concatenate([res.results[cid]["out"] for cid in range(NCORES)],
                         axis=0)
    return out, res


def kernel(data, W, b):
    out, _ = run(data, W, b, trace=False)
    return out
